# revision 3
# baseline (speedup 1.0000x reference)
"""Trainium2 Bass kernel for causal self-attention scores (ContextSelfAttn).

Reference computation (B=2, S=2048, D=1024, H=16, HD=64):
    qk = encoded @ W.T + bias            # [b, s, 2d]
    q, k = split(qk)                     # [b, s, h, hd] each
    scores = einsum('bthd,bshd->bhts', q, k / sqrt(hd))
    scores += causal_mask (-10000 upper triangular)
    out = softmax(scores, axis=-1)       # [b, h, s, s] f32

Sharding: 8 cores = (batch b in {0,1}) x (head-group g in {0..3} of 4 heads).
Each core computes its 4 heads' [2048, 2048] softmaxed score panels.

Per-core device layout (host pre-transposes so no on-chip transposes needed):
  encT  [1024, 2048]  encoded[b].T                      (d on partitions)
  wT    [1024, 512]   W rows for this group, transposed; per head h:
                      cols [h*128:(h+1)*128] = [Wq_h.T (64) | (Wk_h/8).T (64)]
  biasT [4, 128]      per head: [bias_q_h (64) | bias_k_h/8 (64)]
  maskT [4, 128, 512] bf16 {0, -10000}: for tq%4==v, maskT[v][p][c] = -1e4 if c > v*128+p
  ident [128, 128]    bf16 identity (stationary operand for the mask matmul)

Pipeline per core:
  1. qkT[h] [128, 2048] bf16 = W-proj via PE (fp32), bias-add via DVE   (h = head)
     partitions 0-63 = q_h.T, 64-127 = (k_h/8).T
  2. scores psum chunk [128, 512] = qT.T @ kT  (bf16 matmul, K=64);
     diagonal chunks accumulate a second matmul I.T @ maskT = mask add.
  3. ACT: exp(psum) -> SBUF + fused row-sum accum; DVE: reciprocal + scale.
  4. DMA causal part + DMA zeros for the fully-masked tail.
"""

import numpy as np
import ml_dtypes

B, S, D, H = 2, 2048, 1024, 16
HD = D // H          # 64
G = 4                # head groups -> 8 cores = B * G
HPG = H // G         # 4 heads per group
NCORES = 8

TQ = 128             # query rows per panel
TS = 512             # score cols per psum chunk
NTQ = S // TQ        # 16 panels per head
NTS = S // TS        # 4 chunks per row

# Projection matmul input dtype ("float32" accurate / "bfloat16" fast)
PROJ_DT_NAME = "float32"
# Device writes zeros for fully-masked chunks (True) or host fills them (False)
DEVICE_ZEROS = True

_CACHE = {}


def _build(nc_mod):
    bass, tile, bacc, mybir = nc_mod
    f32 = mybir.dt.float32
    bf16 = mybir.dt.bfloat16
    proj_dt = getattr(mybir.dt, PROJ_DT_NAME)
    AF = mybir.ActivationFunctionType
    ND = D // 128     # 8 d-tiles

    nc = bacc.Bacc("TRN2", target_bir_lowering=False, debug=False)
    encT = nc.dram_tensor("encT", [D, S], proj_dt, kind="ExternalInput").ap()
    wT = nc.dram_tensor("wT", [D, HPG * 128], proj_dt, kind="ExternalInput").ap()
    biasT = nc.dram_tensor("biasT", [HPG, 128], f32, kind="ExternalInput").ap()
    maskT = nc.dram_tensor("maskT", [NTS, 128, TS], bf16, kind="ExternalInput").ap()
    ident = nc.dram_tensor("ident", [128, 128], bf16, kind="ExternalInput").ap()
    out = nc.dram_tensor("out", [HPG, S, S], f32, kind="ExternalOutput").ap()

    with tile.TileContext(nc) as tc:
        from contextlib import ExitStack
        with ExitStack() as ctx:
            consts = ctx.enter_context(tc.tile_pool(name="consts", bufs=1))
            qk_pool = ctx.enter_context(tc.tile_pool(name="qk", bufs=1))
            ps_proj = ctx.enter_context(
                tc.tile_pool(name="ps_proj", bufs=2, space="PSUM"))
            ps_sc = ctx.enter_context(
                tc.tile_pool(name="ps_sc", bufs=4, space="PSUM"))
            exps = ctx.enter_context(tc.tile_pool(name="exps", bufs=3))
            outs = ctx.enter_context(tc.tile_pool(name="outs", bufs=4))
            accs = ctx.enter_context(tc.tile_pool(name="accs", bufs=10))

            # ---- constants / inputs ----
            enc_sb = []
            for dt_i in range(ND):
                enc_t = consts.tile([128, S], proj_dt, name=f"enc_{dt_i}")
                nc.sync.dma_start(out=enc_t, in_=encT[dt_i * 128:(dt_i + 1) * 128, :])
                enc_sb.append(enc_t)
            w_sb = []
            for dt_i in range(ND):
                w_t = consts.tile([128, HPG * 128], proj_dt, name=f"w_{dt_i}")
                nc.sync.dma_start(out=w_t, in_=wT[dt_i * 128:(dt_i + 1) * 128, :])
                w_sb.append(w_t)
            bias_sb = consts.tile([128, HPG], f32)
            nc.sync.dma_start(out=bias_sb, in_=biasT.rearrange("h p -> p h"))
            mask_sb = consts.tile([128, NTS, TS], bf16)
            nc.sync.dma_start(out=mask_sb, in_=maskT.rearrange("v p c -> p v c"))
            ident_sb = consts.tile([128, 128], bf16)
            nc.sync.dma_start(out=ident_sb, in_=ident)
            zero_sb = consts.tile([128, S - TS], f32)
            nc.vector.memset(zero_sb, 0.0)

            qkT = [qk_pool.tile([128, S], bf16, name=f"qkT_{h}") for h in range(HPG)]
            # k.T copies re-based at partition 0 (matmul needs lhsT/rhs at the
            # same base partition; DVE cannot shift partitions, DMA can)
            kT = [qk_pool.tile([64, S], bf16, name=f"kT_{h}") for h in range(HPG)]

            for h in range(HPG):
                # ---- projection for head h: qkT[h][e, t] over 4 t-chunks ----
                for t4 in range(NTS):
                    ps = ps_proj.tile([128, TS], f32, tag="ps_proj")
                    for dt_i in range(ND):
                        nc.tensor.matmul(
                            ps,
                            lhsT=w_sb[dt_i][:, h * 128:(h + 1) * 128],
                            rhs=enc_sb[dt_i][:, t4 * TS:(t4 + 1) * TS],
                            start=(dt_i == 0),
                            stop=(dt_i == ND - 1),
                        )
                    nc.vector.tensor_scalar_add(
                        out=qkT[h][:, t4 * TS:(t4 + 1) * TS],
                        in0=ps,
                        scalar1=bias_sb[:, h:h + 1],
                    )
                    nc.sync.dma_start(
                        out=kT[h][:, t4 * TS:(t4 + 1) * TS],
                        in_=qkT[h][64:128, t4 * TS:(t4 + 1) * TS],
                    )

                # ---- scores + softmax for head h ----
                for tq in range(NTQ):
                    nW = tq // NTS + 1       # causal 512-chunks
                    v = tq % NTS             # diagonal sub-position
                    exp_t = exps.tile([128, S], f32, tag="exp")
                    acc = accs.tile([128, NTS], f32, tag="acc")
                    for j in range(nW):
                        diag = (j == nW - 1)
                        ps = ps_sc.tile([128, TS], f32, tag="ps_sc")
                        nc.tensor.matmul(
                            ps,
                            lhsT=qkT[h][0:64, tq * TQ:(tq + 1) * TQ],
                            rhs=kT[h][:, j * TS:(j + 1) * TS],
                            start=True,
                            stop=not diag,
                        )
                        if diag:
                            nc.tensor.matmul(
                                ps,
                                lhsT=ident_sb,
                                rhs=mask_sb[:, v, :],
                                start=False,
                                stop=True,
                            )
                        nc.scalar.activation(
                            out=exp_t[:, j * TS:(j + 1) * TS],
                            in_=ps,
                            func=AF.Exp,
                            accum_out=acc[:, j:j + 1],
                        )
                    rrec = accs.tile([128, 1], f32, tag="rrec")
                    if nW > 1:
                        rsum = accs.tile([128, 1], f32, tag="rsum")
                        nc.vector.tensor_reduce(
                            out=rsum, in_=acc[:, 0:nW],
                            axis=mybir.AxisListType.X, op=mybir.AluOpType.add)
                        nc.vector.reciprocal(out=rrec, in_=rsum)
                    else:
                        nc.vector.reciprocal(out=rrec, in_=acc[:, 0:1])
                    out_t = outs.tile([128, S], f32, tag="out")
                    nc.vector.tensor_scalar_mul(
                        out=out_t[:, 0:nW * TS],
                        in0=exp_t[:, 0:nW * TS],
                        scalar1=rrec,
                    )
                    nc.sync.dma_start(
                        out=out[h, tq * TQ:(tq + 1) * TQ, 0:nW * TS],
                        in_=out_t[:, 0:nW * TS],
                    )
                    if DEVICE_ZEROS and nW < NTS:
                        nc.scalar.dma_start(
                            out=out[h, tq * TQ:(tq + 1) * TQ, nW * TS:S],
                            in_=zero_sb[:, 0:S - nW * TS],
                        )
    nc.compile()
    return nc


def _get_nc():
    if "nc" not in _CACHE:
        import concourse.bass as bass
        import concourse.tile as tile
        from concourse import bacc, mybir
        _CACHE["nc"] = _build((bass, tile, bacc, mybir))
    return _CACHE["nc"]


def _prep_inputs(encoded, W, bias):
    """Host-side sharding: returns in_maps for the 8 cores."""
    np_proj = np.float32 if PROJ_DT_NAME == "float32" else ml_dtypes.bfloat16
    scale = np.float32(1.0 / np.sqrt(HD))

    # masks: maskT[v][p][c] = -10000 if c > v*128 + p
    p_idx = np.arange(128)[:, None]
    c_idx = np.arange(TS)[None, :]
    maskT = np.zeros((NTS, 128, TS), np.float32)
    for v in range(NTS):
        maskT[v] = np.where(c_idx > v * TQ + p_idx, -10000.0, 0.0)
    maskT = maskT.astype(ml_dtypes.bfloat16)
    ident = np.eye(128, dtype=ml_dtypes.bfloat16)

    in_maps = []
    for core in range(NCORES):
        b, g = divmod(core, G)
        encT = np.ascontiguousarray(encoded[b].T).astype(np_proj)
        # per head h (global g*4+h): [Wq rows (64) | Wk rows scaled (64)]
        w_cols = []
        bias_cols = []
        for h in range(HPG):
            hh = g * HPG + h
            wq = W[hh * HD:(hh + 1) * HD, :]              # [64, 1024]
            wk = W[D + hh * HD:D + (hh + 1) * HD, :] * scale
            w_cols.append(np.concatenate([wq, wk], axis=0))   # [128, 1024]
            bq = bias[hh * HD:(hh + 1) * HD]
            bk = bias[D + hh * HD:D + (hh + 1) * HD] * scale
            bias_cols.append(np.concatenate([bq, bk]))    # [128]
        wT = np.ascontiguousarray(
            np.concatenate(w_cols, axis=0).T).astype(np_proj)  # [1024, 512]
        biasT = np.stack(bias_cols).astype(np.float32)         # [4, 128]
        in_maps.append({
            "encT": encT, "wT": wT, "biasT": biasT,
            "maskT": maskT, "ident": ident,
        })
    return in_maps


def kernel(encoded, W, bias):
    from concourse.bass_utils import run_bass_kernel_spmd
    nc = _get_nc()
    in_maps = _prep_inputs(encoded, W, bias)
    res = run_bass_kernel_spmd(nc, in_maps, core_ids=list(range(NCORES)))
    full = np.zeros((B, H, S, S), np.float32)
    for core in range(NCORES):
        b, g = divmod(core, G)
        full[b, g * HPG:(g + 1) * HPG] = res.results[core]["out"]
    return full
